# revision 3
# baseline (speedup 1.0000x reference)
"""Llama4TextMoe (E=8, H=1024, I=4096, top-1) on 8 TRN2 NeuronCores.

Key identity: with TOPK=1 the reference's masked per-expert loop collapses to
    out[t] = SwiGLU_shared(x[t]) + SwiGLU_{expert c}(x[t] * s[t]),  c = t // 256
    s[t] = sigmoid(max_e logit[t,e]);  scores[e,t] = s[t] iff e == argmax else 0
so token chunk c only ever uses expert c's weights -> pure expert/data
parallelism across 8 cores with zero collectives.

Per-core layout strategy (all matmuls transpose-free):
  - x natural [t,h] -> PE-transpose -> xT [h,t] (fp32 for exact router,
    bf16 copy for the MLPs); z = x*s scaled in natural layout, transposed too.
  - gate/up matmuls: weights stationary [128h,128f], xT/zT moving (N=256)
    -> guT [f,t]; silu*up elementwise in [f,t] -> hT [i,t] (bf16)
  - down matmuls: hT stationary [128i,128t], down-weights moving (N=512)
    -> out [t,h] natural; expert + shared accumulate into one PSUM region.
"""

import numpy as np
import ml_dtypes

import concourse.bass as bass
import concourse.mybir as mybir
import concourse.tile as tile
from concourse import bacc
from concourse.bass_utils import run_bass_kernel_spmd
from concourse.masks import make_identity

E, H, I = 8, 1024, 4096
T = 2048            # total tokens (2*1024)
TPC = T // E        # tokens per core = 256
NKH = H // 128      # 8 k-tiles over H
NKI = I // 128      # 32 k-tiles over I
NFT = (2 * I) // 128  # 64 f-tiles over gate+up

MDT = mybir.dt.bfloat16   # MLP matmul dtype
NPD = ml_dtypes.bfloat16  # matching numpy dtype
F32 = mybir.dt.float32

DOWN_LAG = 4  # units between producing hT and consuming it in down matmuls


def _build():
    nc = bacc.Bacc()

    x_d = nc.dram_tensor("x", [TPC, H], F32, kind="ExternalInput")
    rw_d = nc.dram_tensor("rw", [128, NKH, E], F32, kind="ExternalInput")
    # [p, ft, kk, j]: W[kk*128+p, ft*128+j] for the gate_up / shared gate+up weights
    gup_d = nc.dram_tensor("gup", [128, NFT, NKH, 128], MDT, kind="ExternalInput")
    wgu_d = nc.dram_tensor("wgu", [128, NFT, NKH, 128], MDT, kind="ExternalInput")
    down_d = nc.dram_tensor("down", [NKI, 128, H], MDT, kind="ExternalInput")
    wd_d = nc.dram_tensor("wd", [NKI, 128, H], MDT, kind="ExternalInput")
    out_d = nc.dram_tensor("out", [TPC, H], F32, kind="ExternalOutput")
    sc_d = nc.dram_tensor("scores", [E, TPC], F32, kind="ExternalOutput")

    with tile.TileContext(nc) as tc:
        with (
            tc.tile_pool(name="persist", bufs=1) as pp,
            tc.tile_pool(name="stream", bufs=1) as ws,
            tc.tile_pool(name="ps_gu", bufs=2, space="PSUM") as ps_gu,
        ):
            ident = pp.tile([128, 128], F32, tag="ident")
            make_identity(nc, ident)

            rw_sb = pp.tile([128, NKH, E], F32, tag="rw")
            nc.sync.dma_start(out=rw_sb, in_=rw_d[:, :, :])

            xn = []
            for tb in range(2):
                t_ = pp.tile([128, H], F32, tag=f"xn{tb}")
                nc.sync.dma_start(out=t_, in_=x_d[tb * 128:(tb + 1) * 128, :])
                xn.append(t_)

            xT = pp.tile([128, NKH, 2 * 128], F32, tag="xT")
            xTb = pp.tile([128, NKH, 2 * 128], MDT, tag="xTb")
            zTb = pp.tile([128, NKH, 2 * 128], MDT, tag="zTb")
            scT = pp.tile([E, TPC], F32, tag="scT")
            out_sb = [pp.tile([128, H], F32, tag=f"osb{tb}", name=f"osb{tb}")
                      for tb in range(2)]

            # ---- per-unit emission helpers -------------------------------
            # unit u: 0..31 shared (xTb, wgu_d, wd_d), 32..63 expert (zTb, gup_d, down_d)
            hts = {}

            def emit_unit(u):
                sh = u < 32
                k = u if sh else u - 32
                wsrc = wgu_d if sh else gup_d
                xsrc = xTb if sh else zTb
                gt = ws.tile([128, NKH, 128], MDT, tag="wload", bufs=6)
                nc.sync.dma_start(out=gt, in_=wsrc[:, k, :, :])
                ut = ws.tile([128, NKH, 128], MDT, tag="wload", bufs=6)
                nc.sync.dma_start(out=ut, in_=wsrc[:, NKI + k, :, :])
                pg = ps_gu.tile([128, TPC], F32, tag="gate")
                for kk in range(NKH):
                    nc.tensor.matmul(pg, gt[:, kk, :], xsrc[:, kk, :],
                                     start=(kk == 0), stop=(kk == NKH - 1))
                pu = ps_gu.tile([128, TPC], F32, tag="up")
                for kk in range(NKH):
                    nc.tensor.matmul(pu, ut[:, kk, :], xsrc[:, kk, :],
                                     start=(kk == 0), stop=(kk == NKH - 1))
                sl = ws.tile([128, TPC], F32, tag="silu", bufs=3)
                nc.scalar.activation(sl, pg, mybir.ActivationFunctionType.Silu)
                ht = ws.tile([128, TPC], MDT, tag="h", bufs=DOWN_LAG + 4)
                nc.vector.tensor_tensor(ht, sl, pu, op=mybir.AluOpType.mult)
                hts[u] = ht

            def emit_down(u, ps_out):
                sh = u < 32
                k = u if sh else u - 32
                ht = hts.pop(u)
                wdt = ws.tile([128, H], MDT, tag="wdload", bufs=4)
                nc.sync.dma_start(out=wdt, in_=(wd_d if sh else down_d)[k, :, :])
                for tb in range(2):
                    for nf in range(2):
                        nc.tensor.matmul(
                            ps_out[tb][:, nf * 512:(nf + 1) * 512],
                            ht[:, tb * 128:(tb + 1) * 128],
                            wdt[:, nf * 512:(nf + 1) * 512],
                            start=(u == 0), stop=(u == 63))

            # ---- prologue (transposes / router / scores / z) -------------
            with tc.tile_pool(name="ps_pro", bufs=2, space="PSUM") as ps_pro:
                for tb in range(2):
                    for k in range(NKH):
                        pt = ps_pro.tile([128, 128], F32, tag="xp")
                        nc.tensor.transpose(pt, xn[tb][:, k * 128:(k + 1) * 128], ident)
                        nc.vector.tensor_copy(xT[:, k, tb * 128:(tb + 1) * 128], pt)
                        nc.scalar.copy(xTb[:, k, tb * 128:(tb + 1) * 128], pt)

                s_col = []
                for tb in range(2):
                    lg = ps_pro.tile([128, E], F32, tag="lg")
                    for k in range(NKH):
                        nc.tensor.matmul(lg, xT[:, k, tb * 128:(tb + 1) * 128],
                                         rw_sb[:, k, :],
                                         start=(k == 0), stop=(k == NKH - 1))
                    lsb = pp.tile([128, E], F32, tag=f"lsb{tb}")
                    nc.vector.tensor_copy(lsb, lg)
                    m = pp.tile([128, 1], F32, tag=f"m{tb}")
                    nc.vector.reduce_max(out=m, in_=lsb, axis=mybir.AxisListType.X)
                    s = pp.tile([128, 1], F32, tag=f"s{tb}")
                    nc.scalar.activation(s, m, mybir.ActivationFunctionType.Sigmoid)
                    sc = pp.tile([128, E], F32, tag=f"sc{tb}")
                    # (logits == rowmax) * sigmoid(rowmax); non-top lanes exact 0
                    nc.vector.tensor_scalar(sc, lsb, m, s,
                                            op0=mybir.AluOpType.is_equal,
                                            op1=mybir.AluOpType.mult)
                    s_col.append((s, sc))

                # a few shared gu units while scores settle on ACT/DVE
                for u in range(4):
                    emit_unit(u)

                for tb in range(2):
                    s, sc = s_col[tb]
                    # scoresT slab via PE transpose: [128,8] -> [8,128]
                    pt = ps_pro.tile([128, 128], F32, tag="xp")
                    nc.tensor.transpose(pt[:E, :], sc, ident)
                    nc.vector.tensor_copy(scT[:, tb * 128:(tb + 1) * 128], pt[:E, :])
                    # z natural = x * s, then transpose
                    zn = ws.tile([128, H], F32, tag="zn", bufs=2)
                    nc.vector.tensor_scalar_mul(zn, xn[tb], s)
                    for k in range(NKH):
                        pz = ps_pro.tile([128, 128], F32, tag="xp")
                        nc.tensor.transpose(pz, zn[:, k * 128:(k + 1) * 128], ident)
                        nc.scalar.copy(zTb[:, k, tb * 128:(tb + 1) * 128], pz)

                nc.sync.dma_start(out=sc_d[:, :], in_=scT)

            # ---- main: remaining units with lagged down-projections ------
            with tc.tile_pool(name="ps_out", bufs=1, space="PSUM") as ps_o:
                ps_out = [ps_o.tile([128, H], F32, tag=f"out{tb}", name=f"pso{tb}")
                          for tb in range(2)]
                for u in range(4, 64):
                    emit_unit(u)
                    if u - DOWN_LAG >= 0:
                        emit_down(u - DOWN_LAG, ps_out)
                for u in range(64 - DOWN_LAG, 64):
                    emit_down(u, ps_out)

                for tb in range(2):
                    nc.vector.tensor_copy(out_sb[tb], ps_out[tb])
                    nc.sync.dma_start(out=out_d[tb * 128:(tb + 1) * 128, :],
                                      in_=out_sb[tb])

    nc.compile()
    return nc


_NC = None


def _get_nc():
    global _NC
    if _NC is None:
        _NC = _build()
    return _NC


def _prep_core_inputs(x, router_w, gate_up_proj, down_proj,
                      shared_gate_w, shared_up_w, shared_down_w):
    """Host-side shard + layout rearrange (weights cast to bf16)."""
    # router: (H, E) -> [p, k, e]
    rw = np.ascontiguousarray(
        router_w.reshape(NKH, 128, E).transpose(1, 0, 2)).astype(np.float32)
    # shared gate+up merged -> (H, 2I) with gate cols first (like gate_up split)
    wgu_full = np.concatenate([shared_gate_w, shared_up_w], axis=1)  # (H, 2I)
    wgu = np.ascontiguousarray(
        wgu_full.reshape(NKH, 128, NFT, 128).transpose(1, 2, 0, 3)).astype(NPD)
    wd = np.ascontiguousarray(
        shared_down_w.reshape(NKI, 128, H)).astype(NPD)

    maps = []
    for c in range(E):
        gup = np.ascontiguousarray(
            gate_up_proj[c].reshape(NKH, 128, NFT, 128).transpose(1, 2, 0, 3)
        ).astype(NPD)
        dn = np.ascontiguousarray(down_proj[c].reshape(NKI, 128, H)).astype(NPD)
        maps.append({
            "x": np.ascontiguousarray(x[c * TPC:(c + 1) * TPC]).astype(np.float32),
            "rw": rw, "gup": gup, "wgu": wgu, "down": dn, "wd": wd,
        })
    return maps


def kernel(hidden_states, router_w, gate_up_proj, down_proj,
           shared_gate_w, shared_up_w, shared_down_w, _trace=False):
    hidden_states = np.asarray(hidden_states, dtype=np.float32)
    B, S, Hd = hidden_states.shape
    x = hidden_states.reshape(T, H)
    maps = _prep_core_inputs(
        x, np.asarray(router_w, np.float32),
        np.asarray(gate_up_proj, np.float32), np.asarray(down_proj, np.float32),
        np.asarray(shared_gate_w, np.float32), np.asarray(shared_up_w, np.float32),
        np.asarray(shared_down_w, np.float32))

    nc = _get_nc()
    res = run_bass_kernel_spmd(nc, maps, core_ids=list(range(E)), trace=_trace)

    final = np.empty((T, H), np.float32)
    scores_t = np.empty((E, T), np.float32)
    for c in range(E):
        final[c * TPC:(c + 1) * TPC] = res.results[c]["out"]
        scores_t[:, c * TPC:(c + 1) * TPC] = res.results[c]["scores"]

    out = final.reshape(B, S, Hd)
    if _trace:
        kernel._last_results = res
    return out, scores_t


# revision 4
# speedup vs baseline: 1.3199x; 1.3199x over previous
"""Llama4TextMoe (E=8, H=1024, I=4096, top-1) on 8 TRN2 NeuronCores.

Key identity: with TOPK=1 the reference's masked per-expert loop collapses to
    out[t] = SwiGLU_shared(x[t]) + SwiGLU_{expert c}(x[t] * s[t]),  c = t // 256
    s[t] = sigmoid(max_e logit[t,e]);  scores[e,t] = s[t] iff e == argmax else 0
so token chunk c only ever uses expert c's weights -> pure expert/data
parallelism across 8 cores with zero collectives.

Per-core layout strategy (all matmuls transpose-free):
  - x natural [t,h] -> PE-transpose -> xT [h,t] (fp32 for exact router,
    bf16 copy for the MLPs); z = x*s scaled in natural layout, transposed too.
  - gate/up matmuls: weights stationary [128h,128f], xT/zT moving (N=256)
    -> guT [f,t]; silu*up elementwise in [f,t] -> hT [i,t] (bf16)
  - down matmuls: hT stationary [128i,128t], down-weights moving (N=512)
    -> out [t,h] natural; expert + shared accumulate into one PSUM region.
"""

import numpy as np
import ml_dtypes

import concourse.bass as bass
import concourse.mybir as mybir
import concourse.tile as tile
from concourse import bacc
from concourse.bass_utils import run_bass_kernel_spmd
from concourse.masks import make_identity

E, H, I = 8, 1024, 4096
T = 2048            # total tokens (2*1024)
TPC = T // E        # tokens per core = 256
NKH = H // 128      # 8 k-tiles over H
NKI = I // 128      # 32 k-tiles over I
NFT = (2 * I) // 128  # 64 f-tiles over gate+up

MDT = mybir.dt.bfloat16   # MLP matmul dtype
NPD = ml_dtypes.bfloat16  # matching numpy dtype
F32 = mybir.dt.float32

DOWN_LAG = 4  # units between producing hT and consuming it in down matmuls


def _emit_iter(nc, tc, pp, ws, ps_gu, io, it):
    """Emit one full per-core MoE computation (everything except DRAM decls)."""
    x_d, rw_d, gup_d, wgu_d, down_d, wd_d, out_d, sc_d = io
    sfx = f"_{it}"

    ident = pp.tile([128, 128], F32, tag="ident", name="ident" + sfx)
    make_identity(nc, ident)

    rw_sb = pp.tile([128, NKH, E], F32, tag="rw", name="rw" + sfx)
    nc.sync.dma_start(out=rw_sb, in_=rw_d[:, :, :])

    xn = []
    for tb in range(2):
        t_ = pp.tile([128, H], F32, tag=f"xn{tb}", name=f"xn{tb}" + sfx)
        nc.sync.dma_start(out=t_, in_=x_d[tb * 128:(tb + 1) * 128, :])
        xn.append(t_)

    xT = pp.tile([128, NKH, 2 * 128], F32, tag="xT", name="xT" + sfx)
    xTb = pp.tile([128, NKH, 2 * 128], MDT, tag="xTb", name="xTb" + sfx)
    zTb = pp.tile([128, NKH, 2 * 128], MDT, tag="zTb", name="zTb" + sfx)
    scT = pp.tile([E, TPC], F32, tag="scT", name="scT" + sfx)
    out_sb = [pp.tile([128, H], F32, tag=f"osb{tb}", name=f"osb{tb}" + sfx)
              for tb in range(2)]

    # ---- per-unit emission helpers -----------------------------------
    # unit u: 0..31 shared (xTb, wgu_d, wd_d), 32..63 expert (zTb, gup_d, down_d)
    hts = {}

    def emit_unit(u):
        sh = u < 32
        k = u if sh else u - 32
        wsrc = wgu_d if sh else gup_d
        xsrc = xTb if sh else zTb
        gt = ws.tile([128, NKH, 128], MDT, tag="wload", bufs=6,
                     name=f"gt{u}" + sfx)
        nc.sync.dma_start(out=gt, in_=wsrc[:, k, :, :])
        ut = ws.tile([128, NKH, 128], MDT, tag="wload", bufs=6,
                     name=f"ut{u}" + sfx)
        nc.sync.dma_start(out=ut, in_=wsrc[:, NKI + k, :, :])
        pg = ps_gu.tile([128, TPC], F32, tag="gate", name=f"pg{u}" + sfx)
        for kk in range(NKH):
            nc.tensor.matmul(pg, gt[:, kk, :], xsrc[:, kk, :],
                             start=(kk == 0), stop=(kk == NKH - 1))
        pu = ps_gu.tile([128, TPC], F32, tag="up", name=f"pu{u}" + sfx)
        for kk in range(NKH):
            nc.tensor.matmul(pu, ut[:, kk, :], xsrc[:, kk, :],
                             start=(kk == 0), stop=(kk == NKH - 1))
        sl = ws.tile([128, TPC], F32, tag="silu", bufs=3, name=f"sl{u}" + sfx)
        nc.scalar.activation(sl, pg, mybir.ActivationFunctionType.Silu)
        ht = ws.tile([128, TPC], MDT, tag="h", bufs=DOWN_LAG + 4,
                     name=f"ht{u}" + sfx)
        nc.vector.tensor_tensor(ht, sl, pu, op=mybir.AluOpType.mult)
        hts[u] = ht

    def emit_down(u, ps_out):
        sh = u < 32
        k = u if sh else u - 32
        ht = hts.pop(u)
        wdt = ws.tile([128, H], MDT, tag="wdload", bufs=4, name=f"wd{u}" + sfx)
        nc.sync.dma_start(out=wdt, in_=(wd_d if sh else down_d)[k, :, :])
        for tb in range(2):
            for nf in range(2):
                nc.tensor.matmul(
                    ps_out[tb][:, nf * 512:(nf + 1) * 512],
                    ht[:, tb * 128:(tb + 1) * 128],
                    wdt[:, nf * 512:(nf + 1) * 512],
                    start=(u == 0), stop=(u == 63))

    # ---- prologue (transposes / router / scores / z) -----------------
    with tc.tile_pool(name="ps_pro" + sfx, bufs=2, space="PSUM") as ps_pro:
        for tb in range(2):
            for k in range(NKH):
                pt = ps_pro.tile([128, 128], F32, tag="xp", name=f"xp{tb}{k}" + sfx)
                nc.tensor.transpose(pt, xn[tb][:, k * 128:(k + 1) * 128], ident)
                nc.vector.tensor_copy(xT[:, k, tb * 128:(tb + 1) * 128], pt)
                nc.scalar.copy(xTb[:, k, tb * 128:(tb + 1) * 128], pt)

        s_col = []
        for tb in range(2):
            lg = ps_pro.tile([128, E], F32, tag="lg", name=f"lg{tb}" + sfx)
            for k in range(NKH):
                nc.tensor.matmul(lg, xT[:, k, tb * 128:(tb + 1) * 128],
                                 rw_sb[:, k, :],
                                 start=(k == 0), stop=(k == NKH - 1))
            lsb = pp.tile([128, E], F32, tag=f"lsb{tb}", name=f"lsb{tb}" + sfx)
            nc.vector.tensor_copy(lsb, lg)
            m = pp.tile([128, 1], F32, tag=f"m{tb}", name=f"m{tb}" + sfx)
            nc.vector.reduce_max(out=m, in_=lsb, axis=mybir.AxisListType.X)
            s = pp.tile([128, 1], F32, tag=f"s{tb}", name=f"s{tb}" + sfx)
            nc.scalar.activation(s, m, mybir.ActivationFunctionType.Sigmoid)
            sc = pp.tile([128, E], F32, tag=f"sc{tb}", name=f"sc{tb}" + sfx)
            # (logits == rowmax) * sigmoid(rowmax); non-top lanes exact 0
            nc.vector.tensor_scalar(sc, lsb, m, s,
                                    op0=mybir.AluOpType.is_equal,
                                    op1=mybir.AluOpType.mult)
            s_col.append((s, sc))

        # a few shared gu units while scores settle on ACT/DVE
        for u in range(4):
            emit_unit(u)

        for tb in range(2):
            s, sc = s_col[tb]
            # scoresT slab via PE transpose: [128,8] -> [8,128]
            pt = ps_pro.tile([128, 128], F32, tag="xp", name=f"scp{tb}" + sfx)
            nc.tensor.transpose(pt[:E, :], sc, ident)
            nc.vector.tensor_copy(scT[:, tb * 128:(tb + 1) * 128], pt[:E, :])
            # z natural = x * s, then transpose
            zn = ws.tile([128, H], F32, tag="zn", bufs=2, name=f"zn{tb}" + sfx)
            nc.vector.tensor_scalar_mul(zn, xn[tb], s)
            for k in range(NKH):
                pz = ps_pro.tile([128, 128], F32, tag="xp", name=f"zp{tb}{k}" + sfx)
                nc.tensor.transpose(pz, zn[:, k * 128:(k + 1) * 128], ident)
                nc.scalar.copy(zTb[:, k, tb * 128:(tb + 1) * 128], pz)

        nc.sync.dma_start(out=sc_d[:, :], in_=scT)

    # ---- main: remaining units with lagged down-projections ----------
    with tc.tile_pool(name="ps_out" + sfx, bufs=1, space="PSUM") as ps_o:
        ps_out = [ps_o.tile([128, H], F32, tag=f"out{tb}", name=f"pso{tb}" + sfx)
                  for tb in range(2)]
        for u in range(4, 64):
            emit_unit(u)
            if u - DOWN_LAG >= 0:
                emit_down(u - DOWN_LAG, ps_out)
        for u in range(64 - DOWN_LAG, 64):
            emit_down(u, ps_out)

        for tb in range(2):
            nc.vector.tensor_copy(out_sb[tb], ps_out[tb])
            nc.sync.dma_start(out=out_d[tb * 128:(tb + 1) * 128, :],
                              in_=out_sb[tb])


def _build(niter=1):
    nc = bacc.Bacc()

    x_d = nc.dram_tensor("x", [TPC, H], F32, kind="ExternalInput")
    rw_d = nc.dram_tensor("rw", [128, NKH, E], F32, kind="ExternalInput")
    # [p, ft, kk, j]: W[kk*128+p, ft*128+j] for the gate_up / shared gate+up weights
    gup_d = nc.dram_tensor("gup", [128, NFT, NKH, 128], MDT, kind="ExternalInput")
    wgu_d = nc.dram_tensor("wgu", [128, NFT, NKH, 128], MDT, kind="ExternalInput")
    down_d = nc.dram_tensor("down", [NKI, 128, H], MDT, kind="ExternalInput")
    wd_d = nc.dram_tensor("wd", [NKI, 128, H], MDT, kind="ExternalInput")
    out_d = nc.dram_tensor("out", [TPC, H], F32, kind="ExternalOutput")
    sc_d = nc.dram_tensor("scores", [E, TPC], F32, kind="ExternalOutput")
    io = (x_d, rw_d, gup_d, wgu_d, down_d, wd_d, out_d, sc_d)

    with tile.TileContext(nc) as tc:
        with (
            tc.tile_pool(name="persist", bufs=1) as pp,
            tc.tile_pool(name="stream", bufs=1) as ws,
            tc.tile_pool(name="ps_gu", bufs=2, space="PSUM") as ps_gu,
        ):
            for it in range(niter):
                _emit_iter(nc, tc, pp, ws, ps_gu, io, it)

    nc.compile()
    return nc


_NC = {}


def _get_nc(niter=1):
    if niter not in _NC:
        _NC[niter] = _build(niter)
    return _NC[niter]


def _prep_core_inputs(x, router_w, gate_up_proj, down_proj,
                      shared_gate_w, shared_up_w, shared_down_w):
    """Host-side shard + layout rearrange (weights cast to bf16)."""
    # router: (H, E) -> [p, k, e]
    rw = np.ascontiguousarray(
        router_w.reshape(NKH, 128, E).transpose(1, 0, 2)).astype(np.float32)
    # shared gate+up merged -> (H, 2I) with gate cols first (like gate_up split)
    wgu_full = np.concatenate([shared_gate_w, shared_up_w], axis=1)  # (H, 2I)
    wgu = np.ascontiguousarray(
        wgu_full.reshape(NKH, 128, NFT, 128).transpose(1, 2, 0, 3)).astype(NPD)
    wd = np.ascontiguousarray(
        shared_down_w.reshape(NKI, 128, H)).astype(NPD)

    maps = []
    for c in range(E):
        gup = np.ascontiguousarray(
            gate_up_proj[c].reshape(NKH, 128, NFT, 128).transpose(1, 2, 0, 3)
        ).astype(NPD)
        dn = np.ascontiguousarray(down_proj[c].reshape(NKI, 128, H)).astype(NPD)
        maps.append({
            "x": np.ascontiguousarray(x[c * TPC:(c + 1) * TPC]).astype(np.float32),
            "rw": rw, "gup": gup, "wgu": wgu, "down": dn, "wd": wd,
        })
    return maps


def kernel(hidden_states, router_w, gate_up_proj, down_proj,
           shared_gate_w, shared_up_w, shared_down_w, _trace=False):
    hidden_states = np.asarray(hidden_states, dtype=np.float32)
    B, S, Hd = hidden_states.shape
    x = hidden_states.reshape(T, H)
    maps = _prep_core_inputs(
        x, np.asarray(router_w, np.float32),
        np.asarray(gate_up_proj, np.float32), np.asarray(down_proj, np.float32),
        np.asarray(shared_gate_w, np.float32), np.asarray(shared_up_w, np.float32),
        np.asarray(shared_down_w, np.float32))

    nc = _get_nc()
    res = run_bass_kernel_spmd(nc, maps, core_ids=list(range(E)), trace=_trace)

    final = np.empty((T, H), np.float32)
    scores_t = np.empty((E, T), np.float32)
    for c in range(E):
        final[c * TPC:(c + 1) * TPC] = res.results[c]["out"]
        scores_t[:, c * TPC:(c + 1) * TPC] = res.results[c]["scores"]

    out = final.reshape(B, S, Hd)
    if _trace:
        kernel._last_results = res
    return out, scores_t
